# revision 31
# baseline (speedup 1.0000x reference)
"""Trainium2 Bass kernel for nn_DeconvDft2dLayer.

Math reduction: w is [1, 8], so the padded filter hm1 occupies only row 0 of
the [H, W] grid. Hence fft2(hm1)[k, l] is independent of the row frequency k,
and the combined inverse-filter spectrum gmf[k, l] collapses to a real 1D
spectrum g1d[l] = |W1(l)|^-4 along W only (W1 = length-W FFT of the taps;
the flipped/rolled copies pair into conjugates since w is real). The H-axis
FFT then cancels with its inverse, so the whole layer is a per-row circular
convolution:

    y[b, h, :] = ifft(fft(x[b, h, :]) * g1d)  =  x[b, h, :] @ K

with K the real symmetric [W, W] circulant of ker = ifft(g1d). K is computed
on host from the 8 taps (tiny, data-independent of x) and replicated to all
8 cores; x is sharded over batch (4 images per core) and laid out
W-major per shard so the TensorE contraction runs over W directly.

Device kernel per core: Y[2048, 512] = XT[512, 2048].T @ K[512, 512],
64 accumulating [128x128]@[128x512] matmuls in float32r (FP22-truncated
reads, 1-pass full rate, ~1e-4 relative precision).
"""

import numpy as np

import concourse.mybir as mybir
import concourse.tile as tile
from concourse import bacc, bass_utils

B, H, W = 32, 512, 512
N_CORES = 8
ROWS_PER_CORE = B * H // N_CORES  # 2048
N_CHUNKS = ROWS_PER_CORE // 128   # 16
# m-chunks per load group; each group is ONE merged DMA covering all four
# j row-blocks (7 loads total incl. K -> no DMA-sem-lane reuse stalls).
# Small leading groups minimize latency to the first matmul.
GROUP_CHUNKS = (1, 2, 2, 3, 4, 4)

_nc_cache = None
LAST_RESULTS = None  # BassKernelResults of the most recent run (for test.py)


def _build():
    f32 = mybir.dt.float32
    f32r = mybir.dt.float32r

    nc = bacc.Bacc("TRN2", target_bir_lowering=False, debug=False,
                   num_devices=N_CORES)
    # xt = x-shard transposed on host: xt[n, m] = x_shard[m, n]
    xt_d = nc.dram_tensor("xt", [W, ROWS_PER_CORE], f32r,
                          kind="ExternalInput").ap()
    # only the first 128 rows of the circulant K; the other 3 row-blocks
    # are column rotations of this one, built on-device
    k_d = nc.dram_tensor("k", [128, W], f32r, kind="ExternalInput").ap()
    y_d = nc.dram_tensor("y", [ROWS_PER_CORE, W], f32,
                         kind="ExternalOutput").ap()

    group_cols = [128 * c for c in GROUP_CHUNKS]
    group_off = [128 * sum(GROUP_CHUNKS[:g]) for g in range(len(GROUP_CHUNKS))]

    with tile.TileContext(nc) as tc:
        with tc.tile_pool(name="const", bufs=1) as cpool, \
             tc.tile_pool(name="xtp", bufs=1) as xtpool, \
             tc.tile_pool(name="yout", bufs=6) as ypool, \
             tc.tile_pool(name="pyp", bufs=6, space="PSUM") as pypool:
            kts = []
            for j in range(4):
                kt = cpool.tile([128, W], f32r, name=f"kt{j}", tag=f"kt{j}")
                kts.append(kt)
            # K rides the ACT ring (idle until stores begin) so it lands in
            # parallel with the first xt group on the SP ring
            nc.scalar.dma_start(kts[0], k_d)

            # PE warm-up: ~4.5us of dummy matmuls starting right after
            # instruction fetch (~5us), well before the first real matmul
            # (~11us), so the HAM clock gate opens (1.2 -> 2.4 GHz) and the
            # real matmul stream runs warm from its first instruction.
            warm_in = cpool.tile([128, 256], mybir.dt.bfloat16,
                                 name="warm_in")
            nc.vector.memset(warm_in, 0.0)
            warm_ps = pypool.tile([128, 256], f32, name="warm_ps",
                                  tag="warm", bufs=1)
            for r in range(14):
                nc.tensor.matmul(warm_ps, warm_in[:, 0:128],
                                 warm_in, start=(r == 0), stop=(r == 13))
            # K circulant: K[128j+p, q] = K[p, (q - 128j) mod W]
            for j in range(1, 4):
                s = 128 * j
                nc.vector.tensor_copy(kts[j][:, s:W], kts[0][:, 0:W - s])
                nc.vector.tensor_copy(kts[j][:, 0:s], kts[0][:, W - s:W])

            # X^T resident in SBUF as one tile per (row-block j, m-group g)
            # so group loads never WAR-serialize against matmul reads.
            # All loads are issued before any compute: the 8 DMA-completion
            # sem lanes are round-robined over every DMA, and each lane is a
            # serial chain — loads must head the chains or they end up
            # queued behind stores (which wait on compute).
            # One merged DMA per group: tile free layout [j * gc + c],
            # source AP reordered so all 4 j row-blocks load in one shot.
            xtgs = []
            for g, (gc, go) in enumerate(zip(group_cols, group_off)):
                t = xtpool.tile([128, 4 * gc], f32r, name=f"xtg{g}",
                                tag=f"xtg{g}")
                src = xt_d[:, go:go + gc].rearrange("(j p) c -> p j c", j=4)
                nc.sync.dma_start(t.rearrange("p (j c) -> p j c", j=4), src)
                xtgs.append(t)

            yo_pair = None
            for g, (nchunks, go) in enumerate(zip(GROUP_CHUNKS, group_off)):
                xtg = xtgs[g]
                gc = group_cols[g]
                for ci in range(nchunks):
                    i = go // 128 + ci
                    py = pypool.tile([128, W], f32, name=f"py{i}", tag="py")
                    for j in range(4):
                        nc.tensor.matmul(
                            py,
                            xtg[:, j * gc + 128 * ci:j * gc + 128 * (ci + 1)],
                            kts[j],
                            start=(j == 0), stop=(j == 3))
                    # chunk pairs share one [128, 1024] output tile and one
                    # 512KB store; copies alternate DVE/ACT per pair
                    copy_eng = (nc.vector.tensor_copy if (i // 2) % 2
                                else nc.scalar.copy)
                    if i % 2 == 0:
                        yo_pair = ypool.tile([128, 2 * W], f32,
                                             name=f"yo{i // 2}", tag="yo")
                        copy_eng(yo_pair[:, 0:W], py)
                    else:
                        copy_eng(yo_pair[:, W:2 * W], py)
                        # stores ride the ACT HWDGE ring so loads (SP ring)
                        # never queue behind them
                        nc.scalar.dma_start(
                            y_d[128 * (i - 1):128 * (i + 1), :]
                            .rearrange("(c p) q -> p c q", c=2),
                            yo_pair.rearrange("p (c q) -> p c q", c=2))

    nc.compile()
    return nc


def _filter_matrix(w: np.ndarray) -> np.ndarray:
    """[W, W] circulant K with K[n, q] = ker[(q - n) mod W]."""
    taps = np.asarray(w, np.float64).reshape(-1)
    W1 = np.fft.fft(np.pad(taps, (0, W - taps.shape[0])))
    g1d = 1.0 / (np.abs(W1) ** 4)
    ker = np.fft.ifft(g1d).real
    n = np.arange(W)
    return np.ascontiguousarray(
        ker[(n[None, :] - n[:, None]) % W].astype(np.float32))


def kernel(x, w) -> np.ndarray:
    global _nc_cache, LAST_RESULTS
    if _nc_cache is None:
        _nc_cache = _build()
    nc = _nc_cache

    K = np.ascontiguousarray(_filter_matrix(np.asarray(w))[:128])
    xf = np.asarray(x, np.float32).reshape(N_CORES, ROWS_PER_CORE, W)
    in_maps = [{"xt": np.ascontiguousarray(xf[c].T), "k": K}
               for c in range(N_CORES)]
    res = bass_utils.run_bass_kernel_spmd(nc, in_maps,
                                          core_ids=list(range(N_CORES)))
    LAST_RESULTS = res
    y = np.concatenate([r["y"] for r in res.results], axis=0)
    return y.reshape(B, H, W, 1)


# revision 32
# speedup vs baseline: 1.1336x; 1.1336x over previous
"""Trainium2 Bass kernel for nn_DeconvDft2dLayer.

Math reduction: w is [1, 8], so the padded filter hm1 occupies only row 0 of
the [H, W] grid. Hence fft2(hm1)[k, l] is independent of the row frequency k,
and the combined inverse-filter spectrum gmf[k, l] collapses to a real 1D
spectrum g1d[l] = |W1(l)|^-4 along W only (W1 = length-W FFT of the taps;
the flipped/rolled copies pair into conjugates since w is real). The H-axis
FFT then cancels with its inverse, so the whole layer is a per-row circular
convolution:

    y[b, h, :] = ifft(fft(x[b, h, :]) * g1d)  =  x[b, h, :] @ K

with K the real symmetric [W, W] circulant of ker = ifft(g1d). K is computed
on host from the 8 taps (tiny, data-independent of x) and replicated to all
8 cores; x is sharded over batch (4 images per core) and laid out
W-major per shard so the TensorE contraction runs over W directly.

Device kernel per core: Y[2048, 512] = XT[512, 2048].T @ K[512, 512],
64 accumulating [128x128]@[128x512] matmuls in float32r (FP22-truncated
reads, 1-pass full rate, ~1e-4 relative precision).
"""

import numpy as np

import concourse.mybir as mybir
import concourse.tile as tile
from concourse import bacc, bass_utils

B, H, W = 32, 512, 512
N_CORES = 8
ROWS_PER_CORE = B * H // N_CORES  # 2048
N_CHUNKS = ROWS_PER_CORE // 128   # 16
# m-chunks per load group; each group is ONE merged DMA covering all four
# j row-blocks (7 loads total incl. K -> no DMA-sem-lane reuse stalls).
# Small leading groups minimize latency to the first matmul.
GROUP_CHUNKS = (1, 2, 2, 3, 4, 4)

_nc_cache = None
LAST_RESULTS = None  # BassKernelResults of the most recent run (for test.py)


def _build():
    f32 = mybir.dt.float32
    f32r = mybir.dt.float32r

    nc = bacc.Bacc("TRN2", target_bir_lowering=False, debug=False,
                   num_devices=N_CORES)
    # xt = x-shard transposed on host: xt[n, m] = x_shard[m, n]
    xt_d = nc.dram_tensor("xt", [W, ROWS_PER_CORE], f32r,
                          kind="ExternalInput").ap()
    # only the first 128 rows of the circulant K; the other 3 row-blocks
    # are column rotations of this one, built on-device
    k_d = nc.dram_tensor("k", [128, W], f32r, kind="ExternalInput").ap()
    y_d = nc.dram_tensor("y", [ROWS_PER_CORE, W], f32,
                         kind="ExternalOutput").ap()

    group_cols = [128 * c for c in GROUP_CHUNKS]
    group_off = [128 * sum(GROUP_CHUNKS[:g]) for g in range(len(GROUP_CHUNKS))]

    with tile.TileContext(nc) as tc:
        with tc.tile_pool(name="const", bufs=1) as cpool, \
             tc.tile_pool(name="xtp", bufs=1) as xtpool, \
             tc.tile_pool(name="yout", bufs=6) as ypool, \
             tc.tile_pool(name="pyp", bufs=6, space="PSUM") as pypool:
            kts = []
            for j in range(4):
                kt = cpool.tile([128, W], f32r, name=f"kt{j}", tag=f"kt{j}")
                kts.append(kt)
            # K rides the ACT ring (idle until stores begin) so it lands in
            # parallel with the first xt group on the SP ring
            nc.scalar.dma_start(kts[0], k_d)
            # K circulant: K[128j+p, q] = K[p, (q - 128j) mod W]
            for j in range(1, 4):
                s = 128 * j
                nc.vector.tensor_copy(kts[j][:, s:W], kts[0][:, 0:W - s])
                nc.vector.tensor_copy(kts[j][:, 0:s], kts[0][:, W - s:W])

            # X^T resident in SBUF as one tile per (row-block j, m-group g)
            # so group loads never WAR-serialize against matmul reads.
            # All loads are issued before any compute: the 8 DMA-completion
            # sem lanes are round-robined over every DMA, and each lane is a
            # serial chain — loads must head the chains or they end up
            # queued behind stores (which wait on compute).
            # One merged DMA per group: tile free layout [j * gc + c],
            # source AP reordered so all 4 j row-blocks load in one shot.
            xtgs = []
            for g, (gc, go) in enumerate(zip(group_cols, group_off)):
                t = xtpool.tile([128, 4 * gc], f32r, name=f"xtg{g}",
                                tag=f"xtg{g}")
                src = xt_d[:, go:go + gc].rearrange("(j p) c -> p j c", j=4)
                nc.sync.dma_start(t.rearrange("p (j c) -> p j c", j=4), src)
                xtgs.append(t)

            yo_pair = None
            for g, (nchunks, go) in enumerate(zip(GROUP_CHUNKS, group_off)):
                xtg = xtgs[g]
                gc = group_cols[g]
                for ci in range(nchunks):
                    i = go // 128 + ci
                    py = pypool.tile([128, W], f32, name=f"py{i}", tag="py")
                    for j in range(4):
                        nc.tensor.matmul(
                            py,
                            xtg[:, j * gc + 128 * ci:j * gc + 128 * (ci + 1)],
                            kts[j],
                            start=(j == 0), stop=(j == 3))
                    # chunk pairs share one [128, 1024] output tile and one
                    # 512KB store; copies alternate DVE/ACT per pair
                    copy_eng = (nc.vector.tensor_copy if (i // 2) % 2
                                else nc.scalar.copy)
                    if i % 2 == 0:
                        yo_pair = ypool.tile([128, 2 * W], f32,
                                             name=f"yo{i // 2}", tag="yo")
                        copy_eng(yo_pair[:, 0:W], py)
                    else:
                        copy_eng(yo_pair[:, W:2 * W], py)
                        # stores ride the ACT HWDGE ring so loads (SP ring)
                        # never queue behind them
                        nc.scalar.dma_start(
                            y_d[128 * (i - 1):128 * (i + 1), :]
                            .rearrange("(c p) q -> p c q", c=2),
                            yo_pair.rearrange("p (c q) -> p c q", c=2))

    nc.compile()
    return nc


def _filter_matrix(w: np.ndarray) -> np.ndarray:
    """[W, W] circulant K with K[n, q] = ker[(q - n) mod W]."""
    taps = np.asarray(w, np.float64).reshape(-1)
    W1 = np.fft.fft(np.pad(taps, (0, W - taps.shape[0])))
    g1d = 1.0 / (np.abs(W1) ** 4)
    ker = np.fft.ifft(g1d).real
    n = np.arange(W)
    return np.ascontiguousarray(
        ker[(n[None, :] - n[:, None]) % W].astype(np.float32))


def kernel(x, w) -> np.ndarray:
    global _nc_cache, LAST_RESULTS
    if _nc_cache is None:
        _nc_cache = _build()
    nc = _nc_cache

    K = np.ascontiguousarray(_filter_matrix(np.asarray(w))[:128])
    xf = np.asarray(x, np.float32).reshape(N_CORES, ROWS_PER_CORE, W)
    in_maps = [{"xt": np.ascontiguousarray(xf[c].T), "k": K}
               for c in range(N_CORES)]
    res = bass_utils.run_bass_kernel_spmd(nc, in_maps,
                                          core_ids=list(range(N_CORES)))
    LAST_RESULTS = res
    y = np.concatenate([r["y"] for r in res.results], axis=0)
    return y.reshape(B, H, W, 1)
